# revision 12
# baseline (speedup 1.0000x reference)
"""DA_conv Trainium2 kernel (nn_DA_conv_64312840291006).

Reference computation (per sample b):
    d = deg[b,:,0,0]                                  # [64]
    h = leaky_relu(d @ W1.T, 0.1)                     # [64]
    k = (h @ W2.T).reshape(64, 3, 3)                  # per-channel 3x3 kernels
    dw = depthwise_conv3x3(feat[b], k, pad=1)         # [64,160,160]
    dw = leaky_relu(dw, 0.1)
    out = Wc @ dw + bc                                # 1x1 conv over channels
    a = leaky_relu(Wd1 @ d, 0.1); att = sigmoid(Wd2 @ a)
    return out + feat[b] * att[:,None,None]

Sharding: pure data-parallel, B=16 over 8 cores -> 2 samples/core.
Per core the 128 SBUF partitions hold (sample, channel) = p = s*64 + c.

Engine split per core:
  PE : 7 of 9 depthwise taps as diagonal-weight float32r matmuls (PSUM
       accumulate), plus the 1x1 conv as a block-diag(WcT, WcT) matmul.
  DVE: remaining 2 taps via scalar_tensor_tensor FMA (first one merges the
       PE PSUM partial), wrap-around boundary-column fixups, and the final
       merge  out = feat*att + conv  reading the conv PSUM directly.
  ACT: LeakyReLU between depthwise and 1x1 conv; small setup ops.

The image lives in SBUF unpadded ([p, rows*160] contiguous, fast DMA); the
3x3 shifts are flat offsets, and the two wrongly-wrapped columns per row are
corrected by 6 cheap strided fixup ops per row-tile.
"""

import json

import numpy as np

import concourse.bass as bass
import concourse.mybir as mybir
import concourse.tile as tile
from concourse.bass_utils import run_bass_kernel_spmd
from concourse.masks import make_identity


# --------------------------------------------------------------------------
# The walrus build here encodes at most ONE sync-wait per instruction
# ("Too many sync wait commands" in setupSyncWait<...> otherwise), while
# Tile's sem-assignment freely attaches several at convergence points.
# Legalize: hoist all-but-one wait onto same-engine EventSemaphore
# instructions inserted immediately before (engines are in-order, so the
# chain is equivalent).
def _legalize_bir_waits(bir_json: bytes) -> bytes:
    m = json.loads(bir_json)
    counter = [0]
    changed = [0]
    for fn in m.get("functions", []):
        for blk in fn.get("blocks", []):
            out = []
            for inst in blk.get("instructions", []):
                si = inst.get("sync_info") or {}
                waits = si.get("on_wait") or []
                if len(waits) > 1 and inst.get("engine") not in (None, "Unassigned"):
                    changed[0] += 1
                    for w in waits[:-1]:
                        counter[0] += 1
                        out.append(
                            {
                                "engine": inst["engine"],
                                "ins": [],
                                "outs": [],
                                "name": f"I-lgl-wait-{counter[0]}",
                                "opcode": "EventSemaphore",
                                "sync_info": {"on_update": [], "on_wait": [w]},
                            }
                        )
                    si["on_wait"] = waits[-1:]
                out.append(inst)
            blk["instructions"] = out
    return json.dumps(m).encode()


def _install_bir_legalizer():
    import concourse.bass_utils as bu
    import concourse.bass2jax as b2j

    if getattr(bu, "_wait_legalizer_installed", False):
        return
    orig = bu.compile_bir_kernel

    def wrapped(bir_json, tmpdir, neff_name="file.neff"):
        return orig(_legalize_bir_waits(bytes(bir_json)), tmpdir, neff_name=neff_name)

    bu.compile_bir_kernel = wrapped
    b2j.compile_bir_kernel = wrapped
    bu._wait_legalizer_installed = True


_install_bir_legalizer()

F32 = mybir.dt.float32
F32R = mybir.dt.float32r
SLOPE = 0.1

B, C, H, W = 16, 64, 160, 160
NCORES = 8
BPC = B // NCORES            # samples per core
P = BPC * C                  # 128 partitions
HW = H * W                   # 25600
RPT = 32                     # rows per tile
NTILES = H // RPT            # 5
TILE_FD = RPT * W            # 5120
SPAN = 1024                  # dw psum span (2 banks)
NSPAN = TILE_FD // SPAN      # 5
IN_FD = 1 + (RPT + 2) * W + 1  # 5442 (guard + 34 rows + guard)

# tap t = (dy+1)*3 + (dx+1); flat read offset in the guarded input tile
DELTA = [(dy + 1) * W + dx + 1 for dy in (-1, 0, 1) for dx in (-1, 0, 1)]
DVE_TAPS = [7, 8]
PE_TAPS = [t for t in range(9) if t not in DVE_TAPS]

MULT = mybir.AluOpType.mult
ADD = mybir.AluOpType.add
MAX = mybir.AluOpType.max
AF = mybir.ActivationFunctionType


def _build(apply_bias: bool) -> bass.Bass:
    nc = bass.Bass()
    feat = nc.dram_tensor("feat", [P, HW], F32R, kind="ExternalInput")
    deg = nc.dram_tensor("deg", [P], F32, kind="ExternalInput")
    w1 = nc.dram_tensor("w1", [64, 64], F32, kind="ExternalInput")
    w2 = nc.dram_tensor("w2", [576, 64], F32, kind="ExternalInput")
    wc = nc.dram_tensor("wc", [64, 64], F32, kind="ExternalInput")
    bc = nc.dram_tensor("bc", [64], F32, kind="ExternalInput")
    wd1 = nc.dram_tensor("wd1", [8, 64], F32, kind="ExternalInput")
    wd2 = nc.dram_tensor("wd2", [64, 8], F32, kind="ExternalInput")
    out = nc.dram_tensor("out", [P, HW], F32, kind="ExternalOutput")

    with TileKernel(nc) as tk:
        tk.emit(feat, deg, w1, w2, wc, bc, wd1, wd2, out, apply_bias)
    return nc


class TileKernel:
    def __init__(self, nc):
        self.nc = nc

    def __enter__(self):
        self.tc = tile.TileContext(self.nc)
        self.tc.__enter__()
        return self

    def __exit__(self, *a):
        return self.tc.__exit__(*a)

    def emit(self, feat, deg, w1, w2, wc, bc, wd1, wd2, out, apply_bias):
        nc, tc = self.nc, self.tc

        with tc.tile_pool(name="consts", bufs=1) as consts:
            self._setup(consts, deg, w1, w2, wc, bc, wd1, wd2)
            self._main(consts, feat, out, apply_bias)

    # ---------------- setup: weights, per-sample kernel-gen, attention ----
    def _setup(self, consts, deg, w1, w2, wc, bc, wd1, wd2):
        nc, tc = self.nc, self.tc

        idn = consts.tile([128, 128], F32)
        make_identity(nc, idn)

        # raw weight loads
        w1s = consts.tile([64, 64], F32)
        nc.sync.dma_start(out=w1s, in_=w1[:, :])
        w2s = consts.tile([126, 5, 64], F32)
        for j in range(4):
            nc.sync.dma_start(out=w2s[:, j, :], in_=w2[126 * j : 126 * (j + 1), :])
        nc.sync.dma_start(out=w2s[0:72, 4, :], in_=w2[504:576, :])
        wcs = consts.tile([64, 64], F32)
        nc.sync.dma_start(out=wcs, in_=wc[:, :])
        wd1s = consts.tile([8, 64], F32)
        nc.sync.dma_start(out=wd1s, in_=wd1[:, :])
        wd2s = consts.tile([64, 8], F32)
        nc.sync.dma_start(out=wd2s, in_=wd2[:, :])

        # d^T [64, 2]: dT[c, s] = deg[s*64 + c]
        dT = consts.tile([64, BPC], F32)
        deg_ap = deg[:]
        dT_src = bass.AP(
            tensor=deg_ap.tensor, offset=deg_ap.offset, ap=[[1, 64], [64, BPC]]
        )
        nc.sync.dma_start(out=dT, in_=dT_src)

        # bc broadcast to both sample blocks: bc_pp[p] = bc[p % 64]
        self.bc_pp = consts.tile([P, 1], F32)
        bc_ap = bc[:]
        bc_src = bass.AP(
            tensor=bc_ap.tensor, offset=bc_ap.offset, ap=[[0, BPC], [1, 64]]
        )
        nc.sync.dma_start(out=self.bc_pp, in_=bc_src)

        with tc.tile_pool(name="spsum", bufs=4, space="PSUM") as spsum:
            # round d^T into fp32r for the kernel-gen matmuls
            dTr = consts.tile([64, BPC], F32R)
            nc.scalar.copy(dTr, dT)

            # f32 zeros staging tile: guards/halos in f32r tiles are zeroed
            # via ACT copy (a rounding op), since DMA/memset cannot produce
            # fp32r-typed outputs acceptable to the bir verifier.
            zeros = consts.tile([128, 1 + W], F32)
            nc.vector.memset(zeros, 0.0)
            self.zeros = zeros

            # transposes (PE): W1T, W2T, WcT, Wd1T, Wd2T
            w1t = consts.tile([64, 64], F32R)
            pt = spsum.tile([64, 64], F32, tag="s")
            nc.tensor.transpose(pt, w1s, idn[0:64, 0:64])
            nc.scalar.copy(w1t, pt)

            # w2t2[i, t, c] = W2[c*9+t, i]: per-tap contiguous lhsT slices
            w2t2 = consts.tile([64, 9, 64], F32R)
            for j in range(5):
                rows = 126 if j < 4 else 72
                ra = rows // 9
                pt2 = spsum.tile([64, 126], F32, tag="s")
                nc.tensor.transpose(
                    pt2[:, 0:rows], w2s[0:rows, j, :], idn[0:rows, 0:rows]
                )
                src3 = pt2[:, 0:rows].rearrange("p (a b) -> p b a", b=9)
                nc.scalar.copy(w2t2[:, :, 14 * j : 14 * j + ra], src3)

            # The ACT Lrelu table is hardwired to slope 0.01 (alpha arg is
            # ignored), so leaky(x) = 0.1x + 0.9 relu(x) is folded into TWO
            # conv matmuls with pre-scaled block-diag weights and a plain
            # (exact) Relu in between.
            ptc = spsum.tile([64, 64], F32, tag="s")
            nc.tensor.transpose(ptc, wcs, idn[0:64, 0:64])
            wcbd01 = consts.tile([128, 128], F32R)
            nc.scalar.copy(wcbd01[:, 0:128], self.zeros[:, 0:128])
            nc.scalar.mul(wcbd01[0:64, 0:64], ptc, SLOPE)
            nc.sync.dma_start(out=wcbd01[64:128, 64:128], in_=wcbd01[0:64, 0:64])
            wcbd09 = consts.tile([128, 128], F32R)
            nc.scalar.copy(wcbd09[:, 0:128], self.zeros[:, 0:128])
            nc.scalar.mul(wcbd09[0:64, 0:64], ptc, 1.0 - SLOPE)
            nc.sync.dma_start(out=wcbd09[64:128, 64:128], in_=wcbd09[0:64, 0:64])
            self.wcbd01, self.wcbd09 = wcbd01, wcbd09

            wd1t = consts.tile([64, 8], F32R)
            pt3 = spsum.tile([64, 8], F32, tag="s")
            nc.tensor.transpose(pt3, wd1s, idn[0:8, 0:8])
            nc.scalar.copy(wd1t, pt3)

            wd2t = consts.tile([8, 64], F32R)
            pt4 = spsum.tile([8, 64], F32, tag="s")
            nc.tensor.transpose(pt4, wd2s, idn[0:64, 0:64])
            nc.scalar.copy(wd2t, pt4)

            # h^T = leaky(W1 @ d^T)   [64, 2]
            ph = spsum.tile([64, BPC], F32, tag="s")
            nc.tensor.matmul(
                ph, lhsT=w1t, rhs=dTr, start=True, stop=True
            )
            h0 = consts.tile([64, BPC], F32)
            nc.scalar.copy(h0, ph)
            hT = consts.tile([64, BPC], F32R)
            nc.vector.scalar_tensor_tensor(
                out=hT, in0=h0, scalar=SLOPE, in1=h0, op0=MULT, op1=MAX
            )

            # kcols[p = s*64+c, t] = k[s, c, t] = sum_i W2[c*9+t, i] h[s, i]
            # pk2[c, t, s]: one N=2 matmul per tap (fp32r psum writes must
            # be 64-bit-granular, so N=1 per-sample matmuls are illegal)
            pk2 = spsum.tile([64, 9, BPC], F32, tag="s")
            for t in range(9):
                nc.tensor.matmul(
                    pk2[:, t, :],
                    lhsT=w2t2[:, t, :],
                    rhs=hT[:, :],
                    start=True,
                    stop=True,
                )
            # kcols[s*64+c, t] = pk2[c, t, s]; sample 1 block needs a
            # cross-partition move (SBUF->SBUF DMA)
            kcols = consts.tile([P, 9], F32)
            nc.scalar.copy(kcols[0:64, :], pk2[:, :, 0])
            ktmp = consts.tile([64, 9], F32)
            nc.scalar.copy(ktmp, pk2[:, :, 1])
            nc.sync.dma_start(out=kcols[64:128, :], in_=ktmp)
            kneg = consts.tile([P, 9], F32)
            nc.vector.tensor_scalar_mul(kneg, kcols, -1.0)
            self.kcols, self.kneg = kcols, kneg

            # diagonal tap-weight matrices dww[t] = diag(kcols[:, t])
            dww = consts.tile([P, 9, 128], F32R)
            for t in range(9):
                nc.vector.tensor_scalar_mul(dww[:, t, :], idn, kcols[:, t : t + 1])
            self.dww = dww

            # channel attention: att = sigmoid(Wd2 @ leaky(Wd1 @ d))
            pa = spsum.tile([8, BPC], F32, tag="s")
            nc.tensor.matmul(
                pa, lhsT=wd1t, rhs=dTr, start=True, stop=True
            )
            a0 = consts.tile([8, BPC], F32)
            nc.scalar.copy(a0, pa)
            aT = consts.tile([8, BPC], F32R)
            nc.vector.scalar_tensor_tensor(
                out=aT, in0=a0, scalar=SLOPE, in1=a0, op0=MULT, op1=MAX
            )

            patt = spsum.tile([64, BPC], F32, tag="s")
            nc.tensor.matmul(
                patt, lhsT=wd2t, rhs=aT[:, :], start=True, stop=True
            )
            att_pp = consts.tile([P, 1], F32)
            nc.scalar.activation(att_pp[0:64, 0:1], patt[:, 0:1], AF.Sigmoid)
            atmp = consts.tile([64, 1], F32)
            nc.scalar.activation(atmp, patt[:, 1:2], AF.Sigmoid)
            nc.sync.dma_start(out=att_pp[64:128, 0:1], in_=atmp)
            self.att_pp = att_pp

    # ---------------- main loop over row tiles ----------------------------
    def _main(self, consts, feat, out, apply_bias):
        nc, tc = self.nc, self.tc

        with (
            tc.tile_pool(name="inp", bufs=3) as inp,
            tc.tile_pool(name="accp", bufs=2) as accp,
            tc.tile_pool(name="outp", bufs=2) as outp,
            tc.tile_pool(name="dwps", bufs=2, space="PSUM") as dwps,
            tc.tile_pool(name="cvps", bufs=2, space="PSUM") as cvps,
        ):
            for rt in range(NTILES):
                self._row_tile(
                    nc, inp, accp, outp, dwps, cvps, feat, out, rt, apply_bias
                )

    def _row_tile(self, nc, inp, accp, outp, dwps, cvps, feat, out, rt, apply_bias):
        r0 = rt * RPT
        it = inp.tile([P, IN_FD], F32R)
        # guarded load: slot q holds image row (r0 - 1 + q); guards are zero
        if rt == 0:
            nc.scalar.copy(it[:, 0 : 1 + W], self.zeros[:, 0 : 1 + W])
            nc.sync.dma_start(out=it[:, 1 + W : IN_FD - 1], in_=feat[:, 0 : (RPT + 1) * W])
            nc.scalar.copy(it[:, IN_FD - 1 : IN_FD], self.zeros[:, 0:1])
        elif rt == NTILES - 1:
            nc.scalar.copy(it[:, 0:1], self.zeros[:, 0:1])
            nc.sync.dma_start(
                out=it[:, 1 : 1 + (RPT + 1) * W], in_=feat[:, (r0 - 1) * W : HW]
            )
            nc.scalar.copy(it[:, 1 + (RPT + 1) * W : IN_FD], self.zeros[:, 0 : 1 + W])
        else:
            nc.scalar.copy(it[:, 0:1], self.zeros[:, 0:1])
            nc.sync.dma_start(
                out=it[:, 1 : IN_FD - 1], in_=feat[:, (r0 - 1) * W : (r0 + RPT + 1) * W]
            )
            nc.scalar.copy(it[:, IN_FD - 1 : IN_FD], self.zeros[:, 0:1])

        acc = accp.tile([P, TILE_FD], F32R)
        for sp in range(NSPAN):
            base = sp * SPAN
            ps = dwps.tile([P, SPAN], F32)
            for ck in range(SPAN // 512):
                cb = base + ck * 512
                for i, t in enumerate(PE_TAPS):
                    nc.tensor.matmul(
                        ps[:, ck * 512 : (ck + 1) * 512],
                        lhsT=self.dww[:, t, :],
                        rhs=it[:, cb + DELTA[t] : cb + DELTA[t] + 512],
                        start=(i == 0),
                        stop=(i == len(PE_TAPS) - 1),
                    )
            # DVE taps; the first merges the PE partial sum out of PSUM
            t0 = DVE_TAPS[0]
            nc.vector.scalar_tensor_tensor(
                out=acc[:, base : base + SPAN],
                in0=it[:, base + DELTA[t0] : base + DELTA[t0] + SPAN].bitcast(F32),
                scalar=self.kcols[:, t0 : t0 + 1],
                in1=ps[:, :],
                op0=MULT,
                op1=ADD,
            )
            for t in DVE_TAPS[1:]:
                nc.vector.scalar_tensor_tensor(
                    out=acc[:, base : base + SPAN],
                    in0=it[:, base + DELTA[t] : base + DELTA[t] + SPAN].bitcast(F32),
                    scalar=self.kcols[:, t : t + 1],
                    in1=acc[:, base : base + SPAN].bitcast(F32),
                    op0=MULT,
                    op1=ADD,
                )

        # wrap-around fixups: subtract the wrongly-added contributions at
        # x=0 (dx=-1 taps) and x=159 (dx=+1 taps).  The wrong read for
        # output (r, 0) of tap (dy,-1) was it[(r+dy+1)*W]; for (r, W-1) of
        # tap (dy,+1) it was it[(dy+r+1)*W + W + 1].
        def col_ap(base_ap, off):
            one = base_ap[:, off : off + 1]
            return bass.AP(
                tensor=one.tensor, offset=one.offset, ap=[one.ap[0], [W, RPT], [1, 1]]
            )

        acc_r = acc.rearrange("p (r x) -> p r x", x=W)
        for t in (0, 3, 6):  # dx = -1
            dy = t // 3 - 1
            nc.vector.scalar_tensor_tensor(
                out=acc_r[:, :, 0:1],
                in0=col_ap(it, (dy + 1) * W).bitcast(F32),
                scalar=self.kneg[:, t : t + 1],
                in1=acc_r[:, :, 0:1].bitcast(F32),
                op0=MULT,
                op1=ADD,
            )
        for t in (2, 5, 8):  # dx = +1
            dy = t // 3 - 1
            nc.vector.scalar_tensor_tensor(
                out=acc_r[:, :, W - 1 : W],
                in0=col_ap(it, (dy + 1) * W + W + 1).bitcast(F32),
                scalar=self.kneg[:, t : t + 1],
                in1=acc_r[:, :, W - 1 : W].bitcast(F32),
                op0=MULT,
                op1=ADD,
            )

        # 1x1 conv of leaky(dw) as 0.1*Wc@dw + 0.9*Wc@relu(dw), + attention
        fin = outp.tile([P, TILE_FD], F32)
        for sp in range(NSPAN):
            base = sp * SPAN
            ps2 = cvps.tile([P, SPAN], F32)
            for ck in range(SPAN // 512):
                nc.tensor.matmul(
                    ps2[:, ck * 512 : (ck + 1) * 512],
                    lhsT=self.wcbd01,
                    rhs=acc[:, base + ck * 512 : base + (ck + 1) * 512],
                    start=True,
                    stop=False,
                )
            nc.scalar.activation(
                acc[:, base : base + SPAN],
                acc[:, base : base + SPAN].bitcast(F32),
                AF.Relu,
            )
            for ck in range(SPAN // 512):
                nc.tensor.matmul(
                    ps2[:, ck * 512 : (ck + 1) * 512],
                    lhsT=self.wcbd09,
                    rhs=acc[:, base + ck * 512 : base + (ck + 1) * 512],
                    start=False,
                    stop=True,
                )
            nc.vector.scalar_tensor_tensor(
                out=fin[:, base : base + SPAN],
                in0=it[:, base + W + 1 : base + W + 1 + SPAN].bitcast(F32),
                scalar=self.att_pp[:, 0:1],
                in1=ps2[:, :],
                op0=MULT,
                op1=ADD,
            )
        if apply_bias:
            nc.scalar.activation(
                fin, fin, AF.Identity, bias=self.bc_pp[:, 0:1], scale=1.0
            )
        nc.sync.dma_start(out=out[:, r0 * W : (r0 + RPT) * W], in_=fin)


# ---------------- host-side entry point -----------------------------------
_CACHE = {}


def _get_nc(apply_bias: bool) -> bass.Bass:
    if apply_bias not in _CACHE:
        _CACHE[apply_bias] = _build(apply_bias)
    return _CACHE[apply_bias]


def round_fp32r(x: np.ndarray) -> np.ndarray:
    """Round fp32 to fp32r (sign + 8 exp + 11 mantissa bits), RNE."""
    b = np.ascontiguousarray(x, dtype=np.float32).view(np.uint32)
    lsb = (b >> np.uint32(12)) & np.uint32(1)
    r = (b + np.uint32(0x7FF) + lsb) & np.uint32(0xFFFFF000)
    return r.view(np.float32)


def make_in_maps(inputs) -> list[dict]:
    feat = round_fp32r(np.ascontiguousarray(inputs["feat"], dtype=np.float32))
    deg = np.ascontiguousarray(inputs["deg"], dtype=np.float32)
    w1 = np.ascontiguousarray(inputs["W1"], dtype=np.float32)
    w2 = np.ascontiguousarray(inputs["W2"], dtype=np.float32)
    wc = np.ascontiguousarray(inputs["Wc"], dtype=np.float32)
    bc = np.ascontiguousarray(inputs["bc"], dtype=np.float32)
    wd1 = np.ascontiguousarray(inputs["Wd1"], dtype=np.float32)
    wd2 = np.ascontiguousarray(inputs["Wd2"], dtype=np.float32)
    in_maps = []
    for i in range(NCORES):
        sl = slice(BPC * i, BPC * (i + 1))
        in_maps.append(
            {
                "feat": np.ascontiguousarray(feat[sl].reshape(P, HW)),
                "deg": np.ascontiguousarray(deg[sl, :, 0, 0].reshape(P)),
                "w1": w1,
                "w2": w2,
                "wc": wc,
                "bc": bc,
                "wd1": wd1,
                "wd2": wd2,
            }
        )
    return in_maps


def run(inputs, trace: bool = False, tmpdir: str | None = None):
    """Run on all 8 cores; returns (out [16,64,160,160], BassKernelResults)."""
    nc = _get_nc(bool(np.any(np.asarray(inputs["bc"]) != 0.0)))
    res = run_bass_kernel_spmd(
        nc, make_in_maps(inputs), list(range(NCORES)), trace=trace, tmpdir=tmpdir
    )
    out = np.concatenate(
        [r["out"].reshape(BPC, C, H, W) for r in res.results], axis=0
    ).astype(np.float32)
    return out, res


def kernel(**inputs) -> np.ndarray:
    return run(inputs)[0]
